# revision 39
# baseline (speedup 1.0000x reference)
"""Trainium2 (8 NeuronCores) kernel for coverage attention — v12.

168964 ns (v5 baseline) -> 100401 ns.  Key changes:
- Main H*H matmul as 3 fp8-e4m3 DoubleRow passes (split-operand
  xh@Whh + xl@Whh + xh@Whl at scales 2^5/2^8), 0.5 cyc/row on the PE:
  1.33x fewer PE cycles than bf16 at bf16-equivalent accuracy.
- The e-dot (vw . tanh) leaves the PE: DVE fused scalar_tensor_tensor
  accumulates g += f_m*vw_m, GPSIMD partition_all_reduce does the fp32
  cross-partition sum (replicated output, partition 0 row-DMA'd into
  e_sb as fp16).
- ACT tanh batched as [128,1024] calls from 2-bank PSUM tiles with
  per-partition bias A.T[ms, b] and scale 2^-13.
- Softmax without max-subtraction (|e| <= ~18 is exp-safe in fp32):
  batches 0..6 batched with the exp split by column-halves; batch 7
  computed straight off partition 0, its last 1024-piece e-dot on the
  (idle) PE via two [1,512] PSUM-bank slices with exp directly from
  PSUM; normalizing muls fanned across DVE/ACT/GPSIMD.
- The last two batches' pieces interleave (b6p1, b7p1, b6p2, b7p2) so
  batch-7's first-piece exp and the batched-softmax halves slot into
  ACT-idle windows instead of stacking on the tail.
- x hi/lo fp8 slabs DMA'd per s-half; 15 warm DoubleRow matmuls
  (sharing the ps_e PSUM slot) hold the PE p-state while the first
  slab lands; matmul passes ordered (xh@Whh, xh@Whl, xl@Whh) and the
  small const DMAs deferred so compute starts one DMA earlier.

Host prep: coverage folded into x via u = Wc[0] @ Wh^{-1} (f64 solve);
A = dec@Ws + bh+bs+bc stays an exact fp32 ACT bias; v_b dropped
(softmax shift-invariant); sum_coverage = cov + a_t on host.

Engine budget (cost model): PE ~87us busy (82us mains floor), ACT
~74us, DVE ~59us, DMA ~49us, GPSIMD ~25us; makespan 100.4us.
"""

import os
import sys

for _p in ("/opt/trn_rl_repo", os.path.expanduser("~/.axon_site/_ro/trn_rl_repo")):
    if os.path.isdir(_p) and _p not in sys.path:
        sys.path.insert(0, _p)

import ml_dtypes
import numpy as np

import concourse.bass as bass
from concourse import bacc
from concourse import bass_isa
import concourse.tile as tile
from concourse import mybir

B, S, H = 64, 2048, 512
N_CORES = 8
BPC = B // N_CORES

FP = mybir.dt.float32
F16 = mybir.dt.float16
BF = mybir.dt.bfloat16
F8 = mybir.dt.float8e4

SLAB = 512
HC = H // 128

KX = 5   # x scale exponent (max |x~| ~5.8 -> *32 = 186 < 240)
KW = 8   # Wh scale exponent (max |Wh| ~0.22 -> *256 = 56 < 240)
DR = mybir.MatmulPerfMode.DoubleRow

# tuning knobs (sim-A/B'd)
N_WARM = 15
WARM_BUFS = 1
LAST_SPLIT = "1024,1024"
PSF_BUFS = 3
B0_SLABS = 2
XPOOL_BUFS = 3
ROWDMA_GPSIMD = 0
SKIP = set()


def build_program(bpc=BPC, s=S):
    nc = bacc.Bacc(None)

    xh = nc.declare_dram_parameter("xh", [bpc * 128, HC * s], F8, isOutput=False)
    xl = nc.declare_dram_parameter("xl", [bpc * 128, HC * s], F8, isOutput=False)
    whh = nc.declare_dram_parameter("whh", [128, HC * H], F8, isOutput=False)
    whl = nc.declare_dram_parameter("whl", [128, HC * H], F8, isOutput=False)
    at = nc.declare_dram_parameter("at", [H, bpc], FP, isOutput=False)
    vwt = nc.declare_dram_parameter("vwt", [128, HC], FP, isOutput=False)
    out_a = nc.declare_dram_parameter("out_a", [bpc, s], FP, isOutput=True)

    last_split = [int(v) for v in LAST_SPLIT.split(",")]
    assert sum(last_split) == s and all(v % SLAB == 0 for v in last_split)
    max_piece = max(max(last_split), s // 2)

    from contextlib import ExitStack
    with tile.TileContext(nc) as tc, ExitStack() as ctx:
        const = ctx.enter_context(tc.tile_pool(name="const", bufs=1))
        xpool = ctx.enter_context(tc.tile_pool(name="xpool", bufs=XPOOL_BUFS))
        fpool = ctx.enter_context(tc.tile_pool(name="fpool", bufs=2))
        gpool = ctx.enter_context(tc.tile_pool(name="gpool", bufs=2))
        epool = ctx.enter_context(tc.tile_pool(name="epool", bufs=2))
        psf_pool = ctx.enter_context(
            tc.tile_pool(name="ps_f", bufs=PSF_BUFS, space="PSUM"))

        # ---------------- preamble ----------------
        wh_sb = {}
        for nm, src in (("h", whh), ("l", whl)):
            t = const.tile([128, HC, H], F8, tag=f"wh{nm}", name=f"wh{nm}_sb")
            nc.sync.dma_start(out=t, in_=src[:, :].rearrange("p (c j) -> p c j", c=HC))
            wh_sb[nm] = t
        at_sb = []
        for m in range(HC):
            t = const.tile([128, bpc], FP, tag=f"at{m}", name=f"at{m}")
            at_sb.append(t)
        vwt_sb = const.tile([128, HC], FP, tag="vwt")
        vwb = nc.declare_dram_parameter("vwb", [128, HC], BF, isOutput=False)
        vwb_sb = const.tile([128, HC], BF, tag="vwb")

        def load_small_consts():
            for m in range(HC):
                nc.sync.dma_start(
                    out=at_sb[m], in_=at[m * 128:(m + 1) * 128, :])
            nc.sync.dma_start(out=vwt_sb, in_=vwt[:, :])
            nc.sync.dma_start(out=vwb_sb, in_=vwb[:, :])
        pse_pool = ctx.enter_context(tc.tile_pool(name="ps_e", bufs=1, space="PSUM"))
        pse_width = last_split[-1]

        e_sb = const.tile([bpc, s], F16, tag="e_sb")

        # warm the PE p-state while the first x slab loads (reuses the
        # ps_e pool slot — disjoint lifetime with the batch-7 e-dot)
        for i in range(N_WARM):
            wt = pse_pool.tile([128, SLAB], FP, tag="ps_e", name="wt")
            nc.tensor.matmul(
                wt,
                wh_sb["h"][:, 0:2, 0:128],
                wh_sb["h"][:, 0:2, 0:SLAB],
                start=True, stop=True, perf_mode=DR,
            )

        # batch-7 per-piece softmax scratch (all on partition 0)
        n_lp = len(last_split)
        p7 = [const.tile([1, max_piece], FP, tag=f"p7{i}", name=f"p7{i}")
              for i in range(n_lp)]
        s7 = [const.tile([1, 1], FP, tag=f"s7{i}", name=f"s7{i}")
              for i in range(n_lp)]
        ssum = const.tile([1, 1], FP, tag="ssum")
        rs7 = const.tile([1, 1], FP, tag="rs7")
        a7 = const.tile([1, s], FP, tag="a7")

        # ---------------- main loop ----------------
        # piece schedule: batch-major for b0..b5, then b6/b7 interleaved
        # (b6p1, b7p1, b6p2, b7p2) so batch-7's piece-1 exp runs two
        # pieces before the end, off the ACT-saturated tail
        sched = [(b, pi) for b in range(bpc - 2) for pi in range(2)]
        sched += [(bpc - 2, 0), (bpc - 1, 0), (bpc - 2, 1), (bpc - 1, 1)]

        xs_all = {}

        def load_x(b):
            xs = {}
            for nm, srcp in (("h", xh), ("l", xl)):
                t = xpool.tile([128, HC, s], F8, tag=f"xs{nm}")
                src_r = srcp[b * 128:(b + 1) * 128, :].rearrange(
                    "p (c ss) -> p c ss", c=HC)
                npc = B0_SLABS if b == 0 else 2
                for pc in range(npc):
                    sl = slice(pc * (s // npc), (pc + 1) * (s // npc))
                    nc.sync.dma_start(out=t[:, :, sl], in_=src_r[:, :, sl])
                xs[nm] = t
            return xs

        pend_exp = []

        def emit_exp(item):
            pi_, er_, plen_ = item
            nc.scalar.activation(
                out=p7[pi_][:, 0:plen_], in_=er_[0:1, 0:plen_],
                func=mybir.ActivationFunctionType.Exp,
                accum_out=s7[pi_],
            )

        pieces_of = lambda b: (last_split if b == bpc - 1 else [s // 2, s // 2])

        for b, pi in sched:
            last = b == bpc - 1
            if b not in xs_all:
                xs_all[b] = load_x(b)
                if b == 0:
                    # small consts (bias rows, vw) ride behind the first
                    # x slabs — not needed until the first ACT/e-chain
                    load_small_consts()
            xs = xs_all[b]

            if last and pi == 0:
                # rows 0..6 cols 0:1024 are all written by now (b6p1 ran
                # before this piece in the interleaved schedule)
                p_sb = const.tile([bpc, s], FP, tag="p_sb")
                esum = const.tile([bpc, 1], FP, tag="esum")
                nc.scalar.activation(
                    out=p_sb[0:bpc - 1, 0:s // 2],
                    in_=e_sb[0:bpc - 1, 0:s // 2],
                    func=mybir.ActivationFunctionType.Exp,
                    accum_out=esum[0:bpc - 1, :],
                )
            if last and pi == 1:
                # rows 0..6 cols 1024:2048 complete after b6p2
                rsum = const.tile([bpc, 1], FP, tag="rsum")
                a_out = const.tile([bpc, s], FP, tag="a_out")
                esum_b = const.tile([bpc, 1], FP, tag="esum_b")
                nc.scalar.activation(
                    out=p_sb[0:bpc - 1, s // 2:s],
                    in_=e_sb[0:bpc - 1, s // 2:s],
                    func=mybir.ActivationFunctionType.Exp,
                    accum_out=esum_b[0:bpc - 1, :],
                )
                nc.vector.tensor_add(
                    esum[0:bpc - 1, :], esum[0:bpc - 1, :],
                    esum_b[0:bpc - 1, :])
                nc.vector.reciprocal(rsum[0:bpc - 1, :], esum[0:bpc - 1, :])
                nc.vector.tensor_scalar_mul(
                    a_out[0:bpc - 1, :], p_sb[0:bpc - 1, :],
                    rsum[0:bpc - 1, :])
                nc.sync.dma_start(
                    out=out_a[0:bpc - 1, :], in_=a_out[0:bpc - 1, :])

            plen = pieces_of(b)[pi]
            pstart = sum(pieces_of(b)[:pi])
            pe_edot = last and pi == len(pieces_of(b)) - 1
            fsave = []
            g_acc = None
            if not pe_edot:
                g_acc = gpool.tile([128, max_piece], BF, tag="g_acc")
            for m in range(HC):
                ms = slice(m * 128, (m + 1) * 128)
                f_m = fpool.tile([128, max_piece], BF, tag=f"f{m}")
                ps = psf_pool.tile([128, max_piece], FP, tag="ps_f")
                for gi in range(plen // SLAB):
                    goff = pstart + gi * SLAB
                    gsl = slice(goff, goff + SLAB)
                    n = 0
                    for xa, wb in ((xs["h"], wh_sb["h"]),
                                   (xs["h"], wh_sb["l"]),
                                   (xs["l"], wh_sb["h"])):
                        for cp in range(HC // 2):
                            nc.tensor.matmul(
                                ps[:, gi * SLAB:(gi + 1) * SLAB],
                                wb[:, 2 * cp:2 * cp + 2, ms],
                                xa[:, 2 * cp:2 * cp + 2, gsl],
                                start=(n == 0),
                                stop=(n == 5),
                                perf_mode=DR,
                            )
                            n += 1
                nc.scalar.activation(
                    out=f_m[:, 0:plen], in_=ps[:, 0:plen],
                    func=mybir.ActivationFunctionType.Tanh,
                    bias=at_sb[m][:, b:b + 1],
                    scale=float(2.0 ** -(KX + KW)),
                )
                if pe_edot:
                    fsave.append(f_m)
                elif m == 0:
                    nc.vector.tensor_scalar_mul(
                        g_acc[:, 0:plen], f_m[:, 0:plen], vwt_sb[:, 0:1])
                else:
                    nc.vector.scalar_tensor_tensor(
                        g_acc[:, 0:plen], f_m[:, 0:plen],
                        vwt_sb[:, m:m + 1], g_acc[:, 0:plen],
                        op0=mybir.AluOpType.mult,
                        op1=mybir.AluOpType.add,
                    )
            if pe_edot:
                ps_e = pse_pool.tile([1, pse_width], FP, tag="ps_e")
                for hh in range(plen // SLAB):
                    hsl = slice(hh * SLAB, (hh + 1) * SLAB)
                    for m in range(HC):
                        nc.tensor.matmul(
                            ps_e[:, hsl],
                            vwb_sb[:, m:m + 1],
                            fsave[m][:, hsl],
                            start=(m == 0),
                            stop=(m == HC - 1),
                        )
                while pend_exp:
                    emit_exp(pend_exp.pop(0))
                nc.scalar.activation(
                    out=p7[pi][:, 0:plen], in_=ps_e[0:1, 0:plen],
                    func=mybir.ActivationFunctionType.Exp,
                    accum_out=s7[pi],
                )
            else:
                er = epool.tile([128, max_piece], F16, tag="er")
                nc.gpsimd.partition_all_reduce(
                    er[:, 0:plen], g_acc[:, 0:plen], 128,
                    bass_isa.ReduceOp.add)
                if not last:
                    eng = nc.gpsimd if ROWDMA_GPSIMD else nc.sync
                    eng.dma_start(
                        out=e_sb[b:b + 1, pstart:pstart + plen],
                        in_=er[0:1, 0:plen])
                else:
                    # defer one piece so the exp never heads the ACT
                    # queue before its all-reduce input is ready
                    pend_exp.append((pi, er, plen))
                    if len(pend_exp) > 1:
                        emit_exp(pend_exp.pop(0))
        while pend_exp:
            emit_exp(pend_exp.pop(0))

        # batch-7 epilogue: combine pieces, normalize, one output DMA
        nc.vector.tensor_add(ssum, s7[0], s7[1])
        for i in range(2, n_lp):
            nc.vector.tensor_add(ssum, ssum, s7[i])
        nc.vector.reciprocal(rs7, ssum)
        pstart = 0
        for pi, plen in enumerate(last_split):
            dst = a7[:, pstart:pstart + plen]
            srcp = p7[pi][:, 0:plen]
            if pi == 0:
                nc.vector.tensor_scalar_mul(dst, srcp, rs7)
            elif pi == 1:
                nc.scalar.mul(dst, srcp, rs7)
            else:
                nc.gpsimd.tensor_scalar_mul(dst, srcp, rs7)
            pstart += plen
        nc.sync.dma_start(out=out_a[bpc - 1:bpc, :], in_=a7)

    return nc


_PROG_CACHE = {}


def _get_program(key=(BPC, S)):
    if key not in _PROG_CACHE:
        nc = build_program(*key)
        nc.finalize()
        _PROG_CACHE[key] = nc
    return _PROG_CACHE[key]


E4 = ml_dtypes.float8_e4m3


def _q8(v, k):
    """RNE-quantize v*2^k to TRN e4m3 (max +-240); returns float32 array
    still in the scaled domain plus the uint8 bit pattern."""
    s = np.float32(2.0 ** k)
    q = np.clip(v * s, -240.0, 240.0).astype(E4)
    return q.astype(np.float32), q.view(np.uint8)


def make_in_maps(encoder_output, decoder_hidden, coverage, Wh, bh, Ws, bs, Wc, bc,
                 v_w, v_b=None):
    f32 = np.float32
    enc = np.asarray(encoder_output, dtype=f32)
    cov = np.asarray(coverage, dtype=f32)
    Wh64 = np.asarray(Wh, dtype=np.float64)
    # u @ Wh == Wc[0] exactly (f64 solve) -> coverage folds into x
    u = np.linalg.solve(Wh64.T, np.asarray(Wc, dtype=np.float64)[0])
    A = (np.asarray(decoder_hidden, dtype=np.float64)
         @ np.asarray(Ws, dtype=np.float64)
         + np.asarray(bh, dtype=np.float64)
         + np.asarray(bs, dtype=np.float64)
         + np.asarray(bc, dtype=np.float64)).astype(f32)  # [B, H]

    Whf = np.asarray(Wh, dtype=f32)
    whh_f, whh_u8 = _q8(Whf, KW)
    whl_f, whl_u8 = _q8(Whf - whh_f / np.float32(2.0 ** KW), KW)

    def chunked(a2d):
        # [128p, 4c * N] layout with [p, c, n] = a2d[c*128+p, n]
        n = a2d.shape[1]
        return np.ascontiguousarray(
            a2d.reshape(HC, 128, n).transpose(1, 0, 2)).reshape(128, HC * n)

    vw = np.asarray(v_w, dtype=f32).reshape(HC, 128)
    shared = {
        "whh": chunked(whh_u8),
        "whl": chunked(whl_u8),
        "vwt": np.ascontiguousarray(vw.T),
        "vwb": np.ascontiguousarray(vw.T).astype(ml_dtypes.bfloat16).view(np.uint16),
    }
    uf = u.astype(f32)
    in_maps = []
    for c in range(N_CORES):
        lo, hi = c * BPC, (c + 1) * BPC
        xf = enc[lo:hi] + cov[lo:hi][:, :, None] * uf          # [bpc, S, H]
        xt = np.ascontiguousarray(xf.transpose(0, 2, 1))       # [bpc, H, S]
        xh_f, xh_u8 = _q8(xt, KX)
        _, xl_u8 = _q8(xt - xh_f / np.float32(2.0 ** KX), KX)

        def xpack(u8):
            # [bpc*128, 4*S] with row b*128+p holding [c, s] = x[c*128+p, s]
            return np.ascontiguousarray(
                u8.reshape(BPC, HC, 128, S).transpose(0, 2, 1, 3)
            ).reshape(BPC * 128, HC * S)

        m = dict(shared)
        m["xh"] = xpack(xh_u8)
        m["xl"] = xpack(xl_u8)
        m["at"] = np.ascontiguousarray(A[lo:hi].T)             # [H, bpc]
        in_maps.append(m)
    return in_maps


def run_spmd(in_maps, trace=False, **kw):
    from concourse.bass_utils import run_bass_kernel_spmd
    nc = _get_program()
    return run_bass_kernel_spmd(nc, in_maps, core_ids=list(range(N_CORES)),
                                trace=trace, **kw)


def kernel(**inputs) -> tuple[np.ndarray, np.ndarray]:
    in_maps = make_in_maps(**inputs)
    res = run_spmd(in_maps)
    a_t = np.concatenate([r["out_a"] for r in res.results], axis=0)
    a_t = a_t.astype(np.float32)
    cov = np.asarray(inputs["coverage"], dtype=np.float32)
    return a_t, cov + a_t
